# revision 1
# baseline (speedup 1.0000x reference)
"""BiAttention (BiDAF-style) Trainium2 kernel.

Full inputs -> shard batch dim over 8 NeuronCores (4 batches each) -> SPMD
Bass/Tile kernel -> gather full output.

Math (per batch), restructured for the hardware (masks are exact {0,1}):
  R'[d,j]   = w_cq[d]*q[j,d] + w_c[d]         (folds w_c, w_cq into rhs)
  sq[j]     = sum_d q[j,d] w_q[d]
  g[j]      = qm[j] * exp(sq[j])              (folds sq + query mask post-exp)
  S0[c,j]   = sum_d c[c,d] R'[d,j]
  en[c,j]   = exp(S0[c,j]) * g[j]             (= qm_j * exp(S[c,j]))
  attn_c2q  = en / sum_j en                   (== reference masked softmax)
  c2q       = (en @ q) / sum_j en             (denominator via ones column)
  mx[c]     = max_j en[c,j]  = exp(masked-max_j S[c,j])
  e2[c]     = cm[c] * mx[c]
  q2c       = (e2 @ c) / sum_c e2             (== reference q2c)
  G         = [c, c2q, c*c2q, c*q2c]
No max-subtraction is needed: |S| <= ~10 for this regime, exp() is safe in f32.

Context tiles are processed in PAIRS (free dim 512) to amortize per-op fixed
costs. Big matmuls run in float32r (TF32-like, 4x faster PE); transposes and
the data path for G's exact columns stay fp32.
"""

import numpy as np

import bass_rust
import concourse.bass as bass
import concourse.mybir as mybir
from concourse.tile import TileContext
from concourse.bass_utils import run_bass_kernel_spmd
from concourse.masks import make_identity

F32 = mybir.dt.float32
F32R = mybir.dt.float32r
AF = mybir.ActivationFunctionType
OP = mybir.AluOpType
AX = mybir.AxisListType

N_CORES = 8
B, C_L, Q_L, D2 = 32, 2048, 256, 256
BPC = B // N_CORES          # batches per core
NP = C_L // 256             # context tile-pairs per batch (pair = 2x128 rows)
NQ = C_L // 512             # context tile-quads per batch (quad = 4x128 rows)
G_W = 4 * D2                # output row width


def _spill_excess_waits(nc, max_waits: int = 1) -> int:
    """The installed walrus rejects >1 sync wait per instruction. Hoist excess
    waits onto same-engine InstNoOp carriers inserted just before."""
    n = 0
    uid = 0
    for f in nc.m.functions:
        for bb in f.blocks:
            out = []
            changed = False
            for inst in bb.instructions:
                si = inst.sync_info
                waits = list(si.on_wait) if si is not None and si.on_wait else []
                if len(waits) > max_waits:
                    head, tail = waits[:-max_waits], waits[-max_waits:]
                    for i in range(0, len(head), max_waits):
                        out.append(
                            mybir.InstNoOp(
                                name=f"I-wspill-{bb.name}-{uid}",
                                engine=inst.engine,
                                ins=[],
                                outs=[],
                                sync_info=bass_rust.SyncInfo(
                                    on_wait=head[i : i + max_waits], on_update=[]
                                ),
                            )
                        )
                        uid += 1
                        n += 1
                    si.on_wait = tail
                    changed = True
                out.append(inst)
            if changed:
                bb.instructions = out
    return n


WORK_BUFS = 5
BATCH_BUFS = 2
PS_PT_BUFS = 2
PS_MM_BUFS = 4


def build_bass():
    nc = bass.Bass()
    ctx_h = nc.declare_dram_parameter("context", [BPC, C_L, D2], F32, isOutput=False)
    cm_h = nc.declare_dram_parameter("context_mask", [BPC, C_L], F32, isOutput=False)
    q_h = nc.declare_dram_parameter("query", [BPC, Q_L, D2], F32, isOutput=False)
    qm_h = nc.declare_dram_parameter("query_mask", [BPC, Q_L], F32, isOutput=False)
    w_h = nc.declare_dram_parameter("W", [3 * D2], F32, isOutput=False)
    g_h = nc.declare_dram_parameter("G", [BPC, C_L, G_W], F32, isOutput=True)

    with TileContext(nc) as tc:
        with (
            tc.tile_pool(name="const", bufs=1) as cpool,
            tc.tile_pool(name="batch", bufs=BATCH_BUFS) as bpool,
            tc.tile_pool(name="cbuf", bufs=2 * NQ + 2) as cpl,
            tc.tile_pool(name="work", bufs=WORK_BUFS) as wpool,
            tc.tile_pool(name="ps_pt", bufs=PS_PT_BUFS, space="PSUM") as ps_pt,
            tc.tile_pool(name="ps_mm", bufs=PS_MM_BUFS, space="PSUM") as ps_mm,
            tc.tile_pool(name="ps_u", bufs=2, space="PSUM") as ps_u,
        ):
            ident = cpool.tile([128, 128], F32)
            make_identity(nc, ident[:])
            ident_r = cpool.tile([128, 128], F32R)
            nc.vector.tensor_copy(ident_r[:], ident[:])
            ones_row = cpool.tile([1, 128], F32)
            nc.vector.memset(ones_row[:], 1.0)
            ones_col = cpool.tile([128, 1], F32)
            nc.vector.memset(ones_col[:], 1.0)
            eps1 = cpool.tile([1, 1], F32)
            nc.vector.memset(eps1[:], 1e-38)
            # W as [128, 6] columns: a=0,1 -> w_c chunks; 2,3 -> w_q; 4,5 -> w_cq
            w6 = cpool.tile([128, 6], F32)
            nc.gpsimd.dma_start(out=w6[:], in_=w_h.rearrange("(a p) -> p a", p=128))

            for b in range(BPC):
                # ---------- per-batch setup ----------
                # q chunks (f32r) with a ones column appended (denominator)
                q_f32 = bpool.tile([128, 2 * D2], F32, tag="q_f32")
                nc.sync.dma_start(
                    out=q_f32[:].rearrange("p (t d) -> p t d", t=2),
                    in_=q_h[b].rearrange("(t p) d -> p t d", p=128),
                )
                q_ext = []
                for jc in range(2):
                    qe = bpool.tile([128, D2 + 4], F32R, tag=f"q_ext{jc}")
                    nc.vector.tensor_copy(qe[:, 0:D2], q_f32[:, jc * D2 : (jc + 1) * D2])
                    nc.vector.memset(qe[:, D2 : D2 + 1].bitcast(F32), 1.0)
                    nc.vector.memset(qe[:, D2 + 1 : D2 + 4].bitcast(F32), 0.0)
                    q_ext.append(qe)

                # qT via 4 PE transposes: qT_sb cols [dc*256, dc*256+256) hold
                # q rows (j) for d-chunk dc
                qT_sb = bpool.tile([128, 2 * Q_L], F32, tag="qT")
                for dc in range(2):
                    qt_ps = ps_pt.tile([128, Q_L], F32, tag="pt")
                    for jc in range(2):
                        nc.tensor.transpose(
                            qt_ps[:, jc * 128 : (jc + 1) * 128],
                            q_f32[:, jc * D2 + dc * 128 : jc * D2 + (dc + 1) * 128],
                            ident[:],
                        )
                    nc.scalar.copy(qT_sb[:, dc * Q_L : (dc + 1) * Q_L], qt_ps[:])

                # R'[dc] = qT*w_cq + w_c (f32r) ; sq = w_q^T @ qT
                Rp = []
                sq_ps = ps_pt.tile([1, Q_L], F32, tag="pt")
                for dc in range(2):
                    rp = bpool.tile([128, Q_L], F32R, tag=f"Rp{dc}")
                    nc.vector.tensor_scalar(
                        out=rp[:],
                        in0=qT_sb[:, dc * Q_L : (dc + 1) * Q_L],
                        scalar1=w6[:, 4 + dc : 5 + dc],
                        scalar2=w6[:, 0 + dc : 1 + dc],
                        op0=OP.mult,
                        op1=OP.add,
                    )
                    Rp.append(rp)
                    nc.tensor.matmul(
                        sq_ps[:],
                        w6[:, 2 + dc : 3 + dc],
                        qT_sb[:, dc * Q_L : (dc + 1) * Q_L],
                        start=(dc == 0),
                        stop=(dc == 1),
                    )

                # lng[j] = sq[j] + ln(qm[j]) (ln(0+1e-38) = -87.5 kills
                # masked columns inside the exp); folded into the S0 psum
                # accumulation as a K=1 matmul, so exp(S0+lng) IS masked en
                qm_row = bpool.tile([1, Q_L], F32, tag="qm_row")
                nc.sync.dma_start(
                    out=qm_row[:], in_=qm_h[b].rearrange("(o j) -> o j", o=1)
                )
                lnqm = bpool.tile([1, Q_L], F32, tag="lnqm")
                nc.scalar.activation(lnqm[:], qm_row[:], AF.Ln, bias=eps1[:])
                lng_row = bpool.tile([1, Q_L], F32, tag="lng_row")
                nc.vector.tensor_add(lng_row[:], sq_ps[:], lnqm[:])

                # context mask as [128, 16] (col i = 128-row block i), E2 (f32r)
                CM = bpool.tile([128, 2 * NP], F32, tag="CM")
                nc.sync.dma_start(
                    out=CM[:], in_=cm_h[b].rearrange("(i p) -> p i", p=128)
                )
                E2 = bpool.tile([128, 2 * NP], F32R, tag="E2")

                u2c_ps = ps_u.tile([1, D2], F32, tag="u2c")

                # ---------- main loop over quads of context tiles ----------
                # quad = 4 x 128 context rows; psum-adjacent work runs at pair
                # granularity (PSUM bank limits), SBUF-only ops and DMAs at
                # quad granularity to amortize fixed per-instruction costs.
                c_quads = []
                for p in range(NQ):
                    R0 = p * 512  # first context row of the quad
                    c_quad = cpl.tile([128, 1024], F32, tag="c")
                    if b == 0 and p == 0:
                        # split the very first load so the pipeline fills faster
                        for hh in range(2):
                            nc.sync.dma_start(
                                out=c_quad[:, hh * 512 : (hh + 1) * 512].rearrange(
                                    "p (t d) -> p t d", t=2
                                ),
                                in_=ctx_h[
                                    b, R0 + hh * 256 : R0 + (hh + 1) * 256, :
                                ].rearrange("(t p) d -> p t d", p=128),
                            )
                    else:
                        nc.sync.dma_start(
                            out=c_quad[:].rearrange("p (t d) -> p t d", t=4),
                            in_=ctx_h[b, R0 : R0 + 512, :].rearrange(
                                "(t p) d -> p t d", p=128
                            ),
                        )
                    c_quads.append(c_quad)
                    # f32r copy of c for the u2c matmul (gpsimd: SBUF->SBUF)
                    c_r = wpool.tile([128, 1024], F32R, tag="c_r")
                    nc.gpsimd.tensor_copy(c_r[:], c_quad[:])

                    en = wpool.tile([128, 1024], F32R, tag="en")
                    c2q_quad = wpool.tile([128, 1024], F32, tag="c2q")

                    for h in range(2):  # half = pair of context tiles
                        H0 = h * 512
                        # cT: 4 PE transposes -> one psum bank -> sbuf (f32r)
                        pt_c = ps_pt.tile([128, 512], F32, tag="pt")
                        for o in range(0, 512, 128):
                            nc.tensor.transpose(
                                pt_c[:, o : o + 128],
                                c_quad[:, H0 + o : H0 + o + 128],
                                ident[:],
                            )
                        cT_sb = wpool.tile([128, 512], F32R, tag="cT")
                        if h == 0:
                            nc.vector.tensor_copy(cT_sb[:], pt_c[:])
                        else:
                            nc.scalar.copy(cT_sb[:], pt_c[:])

                        # S0 for both tiles of the pair into one psum bank
                        s0_ps = ps_mm.tile([128, 512], F32, tag="mm")
                        for t in range(2):
                            for dc in range(2):
                                nc.tensor.matmul(
                                    s0_ps[:, t * 256 : (t + 1) * 256],
                                    cT_sb[
                                        :,
                                        t * 256 + dc * 128 : t * 256 + (dc + 1) * 128,
                                    ],
                                    Rp[dc][:],
                                    start=(dc == 0),
                                    stop=False,
                                )
                            nc.tensor.matmul(
                                s0_ps[:, t * 256 : (t + 1) * 256],
                                ones_row[:],
                                lng_row[:],
                                start=False,
                                stop=True,
                            )
                        nc.scalar.activation(
                            en[:, H0 : H0 + 512], s0_ps[:], AF.Exp
                        )

                    mx = wpool.tile([128, 4], F32, tag="mx")

                    for h in range(2):
                        H0 = h * 512
                        # mx = per-tile max over j (en already masked via lng)
                        nc.vector.tensor_reduce(
                            out=mx[:, 2 * h : 2 * h + 2],
                            in_=en[:, H0 : H0 + 512]
                            .bitcast(F32)
                            .rearrange("p (t j) -> p t j", t=2),
                            axis=AX.X,
                            op=OP.max,
                        )
                        nc.vector.tensor_mul(
                            E2[:, 4 * p + 2 * h : 4 * p + 2 * h + 2],
                            mx[:, 2 * h : 2 * h + 2],
                            CM[:, 4 * p + 2 * h : 4 * p + 2 * h + 2],
                        )
                        # enT: 4 PE transposes -> one psum bank -> sbuf
                        pt_e = ps_pt.tile([128, 512], F32, tag="pt")
                        for o in range(0, 512, 128):
                            nc.tensor.transpose(
                                pt_e[:, o : o + 128].bitcast(F32R),
                                en[:, H0 + o : H0 + o + 128],
                                ident_r[:],
                            )
                        enT_sb = wpool.tile([128, 512], F32R, tag="enT")
                        for tq in range(2):
                            sl = slice(tq * 256, (tq + 1) * 256)
                            nc.scalar.copy(enT_sb[:, sl], pt_e[:, sl])

                        # c2q per tile (+ denominator in last column); alternate
                        # the normalize-copy between ACT and DVE
                        for t in range(2):
                            c2q_ps = ps_mm.tile([128, D2 + 4], F32, tag="mm")
                            for jc in range(2):
                                nc.tensor.matmul(
                                    c2q_ps[:],
                                    enT_sb[
                                        :,
                                        t * 256 + jc * 128 : t * 256 + (jc + 1) * 128,
                                    ],
                                    q_ext[jc][:],
                                    start=(jc == 0),
                                    stop=(jc == 1),
                                )
                            rcp = wpool.tile([128, 1], F32, tag="rcp")
                            nc.vector.reciprocal(rcp[:], c2q_ps[:, D2 : D2 + 1])
                            dst = c2q_quad[:, H0 + t * 256 : H0 + (t + 1) * 256]
                            if t == 0:
                                nc.scalar.activation(
                                    dst, c2q_ps[:, 0:D2], AF.Identity, scale=rcp[:]
                                )
                            else:
                                nc.vector.tensor_scalar_mul(
                                    dst, c2q_ps[:, 0:D2], rcp[:]
                                )

                        # u2c accumulation (q2c numerator), f32r
                        for t in range(2):
                            tt = 2 * h + t
                            nc.tensor.matmul(
                                u2c_ps[:],
                                E2[:, 4 * p + tt : 4 * p + tt + 1],
                                c_r[:, tt * 256 : (tt + 1) * 256],
                                start=(p == 0 and tt == 0),
                                stop=(p == NQ - 1 and tt == 3),
                            )

                    # cc2q on gpsimd; stream out G columns 0..768 as quad DMAs
                    cc2q = wpool.tile([128, 1024], F32, tag="cc2q")
                    for hh in range(2):
                        sl = slice(hh * 512, (hh + 1) * 512)
                        nc.gpsimd.tensor_mul(
                            cc2q[:, sl], c_quad[:, sl], c2q_quad[:, sl]
                        )

                    for col0, srct in ((0, c_quad), (D2, c2q_quad), (2 * D2, cc2q)):
                        nc.sync.dma_start(
                            out=g_h[b, R0 : R0 + 512, col0 : col0 + D2].rearrange(
                                "(t p) d -> p t d", p=128
                            ),
                            in_=srct[:].rearrange("p (t d) -> p t d", t=4),
                        )

                # ---------- q2c + phase 2 ----------
                z2 = bpool.tile([128, 1], F32, tag="z2")
                nc.vector.reduce_sum(z2[:], E2[:].bitcast(F32), axis=AX.X)
                z2_ps = ps_pt.tile([1, 1], F32, tag="pt")
                nc.tensor.matmul(z2_ps[:], z2[:], ones_col[:], start=True, stop=True)
                rz = bpool.tile([1, 1], F32, tag="rz")
                nc.vector.reciprocal(rz[:], z2_ps[:])
                q2c_row = bpool.tile([1, D2], F32, tag="q2c_row")
                nc.vector.tensor_scalar_mul(q2c_row[:], u2c_ps[:], rz[:])
                q2c_ps = ps_pt.tile([128, D2], F32, tag="pt")
                nc.tensor.matmul(
                    q2c_ps[:], ones_row[:], q2c_row[:], start=True, stop=True
                )
                Q2C = bpool.tile([128, D2], F32, tag="Q2C")
                nc.scalar.copy(Q2C[:], q2c_ps[:])

                for p in range(NQ):
                    R0 = p * 512
                    cq2c = wpool.tile([128, 1024], F32, tag="cq2c")
                    (nc.vector if p % 2 == 0 else nc.gpsimd).tensor_mul(
                        cq2c[:].rearrange("p (t d) -> p t d", t=4),
                        c_quads[p][:].rearrange("p (t d) -> p t d", t=4),
                        Q2C[:].rearrange("p (o d) -> p o d", o=1).broadcast_to(
                            [128, 4, D2]
                        ),
                    )
                    nc.sync.dma_start(
                        out=g_h[b, R0 : R0 + 512, 3 * D2 : 4 * D2].rearrange(
                            "(t p) d -> p t d", p=128
                        ),
                        in_=cq2c[:].rearrange("p (t d) -> p t d", t=4),
                    )

    _spill_excess_waits(nc)
    return nc


_NC_CACHE = None


def _get_nc():
    global _NC_CACHE
    if _NC_CACHE is None:
        _NC_CACHE = build_bass()
    return _NC_CACHE


def kernel(**inputs) -> np.ndarray:
    ctx = np.ascontiguousarray(np.asarray(inputs["context"], dtype=np.float32))
    cm = np.ascontiguousarray(np.asarray(inputs["context_mask"], dtype=np.float32))
    q = np.ascontiguousarray(np.asarray(inputs["query"], dtype=np.float32))
    qm = np.ascontiguousarray(np.asarray(inputs["query_mask"], dtype=np.float32))
    w = np.ascontiguousarray(np.asarray(inputs["W"], dtype=np.float32))

    in_maps = []
    for core in range(N_CORES):
        lo, hi = core * BPC, (core + 1) * BPC
        in_maps.append(
            {
                "context": ctx[lo:hi],
                "context_mask": cm[lo:hi],
                "query": q[lo:hi],
                "query_mask": qm[lo:hi],
                "W": w,
            }
        )

    nc = _get_nc()
    res = run_bass_kernel_spmd(nc, in_maps, list(range(N_CORES)))
    return np.concatenate([res.results[i]["G"] for i in range(N_CORES)], axis=0)



# revision 12
# speedup vs baseline: 1.0210x; 1.0210x over previous
"""BiAttention (BiDAF-style) Trainium2 kernel.

Full inputs -> shard batch dim over 8 NeuronCores (4 batches each) -> SPMD
Bass/Tile kernel -> gather full output.

Math (per batch), restructured for the hardware (masks are exact {0,1}):
  R'[d,j]   = w_cq[d]*q[j,d] + w_c[d]         (folds w_c, w_cq into rhs)
  sq[j]     = sum_d q[j,d] w_q[d]
  lng[j]    = sq[j] + ln(qm[j]+1e-38)         (kills masked cols inside exp)
  S0[c,j]   = sum_d c[c,d] R'[d,j]
  en[c,j]   = exp(S0[c,j] + lng[j])           (= qm_j * exp(S[c,j]))
  attn_c2q  = en / sum_j en                   (== reference masked softmax)
  c2q       = (en @ q) / sum_j en             (denominator via ones column)
  mx[c]     = max_j en[c,j]  = exp(masked-max_j S[c,j])
  e2[c]     = cm[c] * mx[c]
  q2c       = (e2 @ c) / sum_c e2             (== reference q2c)
  G         = [c, c2q, c*c2q, c*q2c]
No max-subtraction is needed: |S| <= ~10 for this regime, exp() is safe in f32.

Schedule: the kernel is DMA-bound (43 MB of HBM traffic per core at the
360 GB/s aggregate DMA rate ~= 120 us). ALL input DMAs are issued up-front
on SP into fully-resident SBUF tiles (ctx 8 MB + q 1 MB fit easily), so the
DMA engines always have queued work while compute pipelines; output writes
then outpace the drain rate and the DMA device stays saturated end-to-end.
Mask/weight loads use contiguous-descriptor layouts + on-chip PE transposes
(a gather-pattern mask load costs 896 ns vs 23 ns contiguous). All matmuls
run f32r (1 cycle/row when the moving dim is >= 256); the lng/q2c broadcast
matmuls were f32 4-pass before. G's exact columns (c, c*c2q, c*q2c) never
pass through f32r.
"""

import numpy as np

import bass_rust
import concourse.bass as bass
import concourse.mybir as mybir
from concourse.tile import TileContext
from concourse.bass_utils import run_bass_kernel_spmd
from concourse.masks import make_identity

F32 = mybir.dt.float32
F32R = mybir.dt.float32r
AF = mybir.ActivationFunctionType
OP = mybir.AluOpType
AX = mybir.AxisListType

N_CORES = 8
B, C_L, Q_L, D2 = 32, 2048, 256, 256
BPC = B // N_CORES          # batches per core
NP = C_L // 256             # context tile-pairs per batch
NQ = C_L // 512             # context tile-quads per batch (quad = 4x128 rows)
G_W = 4 * D2                # output row width


def _spill_excess_waits(nc, max_waits: int = 1) -> int:
    """The installed walrus rejects >1 sync wait per instruction. Hoist excess
    waits onto same-engine InstNoOp carriers inserted just before."""
    n = 0
    uid = 0
    for f in nc.m.functions:
        for bb in f.blocks:
            out = []
            changed = False
            for inst in bb.instructions:
                si = inst.sync_info
                waits = list(si.on_wait) if si is not None and si.on_wait else []
                if len(waits) > max_waits:
                    head, tail = waits[:-max_waits], waits[-max_waits:]
                    for i in range(0, len(head), max_waits):
                        out.append(
                            mybir.InstNoOp(
                                name=f"I-wspill-{bb.name}-{uid}",
                                engine=inst.engine,
                                ins=[],
                                outs=[],
                                sync_info=bass_rust.SyncInfo(
                                    on_wait=head[i : i + max_waits], on_update=[]
                                ),
                            )
                        )
                        uid += 1
                        n += 1
                    si.on_wait = tail
                    changed = True
                out.append(inst)
            if changed:
                bb.instructions = out
    return n


def build_bass():
    nc = bass.Bass()
    ctx_h = nc.declare_dram_parameter("context", [BPC, C_L, D2], F32, isOutput=False)
    cm_h = nc.declare_dram_parameter("context_mask", [BPC, C_L], F32, isOutput=False)
    q_h = nc.declare_dram_parameter("query", [BPC, Q_L, D2], F32, isOutput=False)
    qm_h = nc.declare_dram_parameter("query_mask", [BPC, Q_L], F32, isOutput=False)
    w_h = nc.declare_dram_parameter("W", [3 * D2], F32, isOutput=False)
    g_h = nc.declare_dram_parameter("G", [BPC, C_L, G_W], F32, isOutput=True)

    with TileContext(nc) as tc:
        with (
            tc.tile_pool(name="const", bufs=1) as cpool,
            tc.tile_pool(name="qin", bufs=BPC) as qin,
            tc.tile_pool(name="cbuf", bufs=BPC * NQ) as cpl,
            tc.tile_pool(name="batch", bufs=2) as bpool,
            tc.tile_pool(name="work", bufs=3) as wpool,
            tc.tile_pool(name="ps_pt", bufs=2, space="PSUM") as ps_pt,
            tc.tile_pool(name="ps_mm", bufs=4, space="PSUM") as ps_mm,
            tc.tile_pool(name="ps_u", bufs=2, space="PSUM") as ps_u,
        ):
            # ---------- constants ----------
            ident = cpool.tile([128, 128], F32)
            make_identity(nc, ident[:])
            ident_r = cpool.tile([128, 128], F32R)
            nc.vector.tensor_copy(ident_r[:], ident[:])
            ones_row_r = cpool.tile([1, 128], F32R)
            nc.vector.memset(ones_row_r[:].bitcast(F32), 1.0)
            ones_col = cpool.tile([128, 1], F32)
            nc.vector.memset(ones_col[:], 1.0)
            eps1 = cpool.tile([1, 1], F32)
            nc.vector.memset(eps1[:], 1e-38)

            # ---------- preload: create all input tiles, issue all loads ----
            wT_raw = cpool.tile([6, 128], F32)
            q_raws, qm_rows, cmT_raws = [], [], []
            c_quads = [[None] * NQ for _ in range(BPC)]
            for b in range(BPC):
                q_raws.append(qin.tile([128, 2 * D2], F32, tag="qraw", name=f"qraw{b}"))
                qm_rows.append(qin.tile([1, Q_L], F32, tag="qm", name=f"qm{b}"))
                cmT_raws.append(qin.tile([16, 128], F32, tag="cmT", name=f"cmT{b}"))
                for p in range(NQ):
                    c_quads[b][p] = cpl.tile([128, 1024], F32, tag="c", name=f"c{b}_{p}")

            def load_q(b):
                nc.sync.dma_start(
                    out=q_raws[b][:].rearrange("p (t d) -> p t d", t=2),
                    in_=q_h[b].rearrange("(t p) d -> p t d", p=128),
                )

            def load_qm(b):
                nc.sync.dma_start(
                    out=qm_rows[b][:], in_=qm_h[b].rearrange("(o j) -> o j", o=1)
                )

            def load_cm(b):
                nc.sync.dma_start(
                    out=cmT_raws[b][:], in_=cm_h[b].rearrange("(i p) -> i p", p=128)
                )

            def load_c(b, p, half=None):
                R0 = p * 512
                if half is None:
                    nc.sync.dma_start(
                        out=c_quads[b][p][:].rearrange("p (t d) -> p t d", t=4),
                        in_=ctx_h[b, R0 : R0 + 512, :].rearrange(
                            "(t p) d -> p t d", p=128
                        ),
                    )
                else:
                    nc.sync.dma_start(
                        out=c_quads[b][p][:, half * 512 : (half + 1) * 512].rearrange(
                            "p (t d) -> p t d", t=2
                        ),
                        in_=ctx_h[
                            b, R0 + half * 256 : R0 + (half + 1) * 256, :
                        ].rearrange("(t p) d -> p t d", p=128),
                    )

            # priority order: what unblocks compute earliest goes first
            load_q(0)
            nc.sync.dma_start(out=wT_raw[:], in_=w_h.rearrange("(a p) -> a p", p=128))
            load_c(0, 0, 0)
            load_c(0, 0, 1)
            load_qm(0)
            load_cm(0)
            for p in range(1, NQ):
                load_c(0, p)
            for b in range(1, BPC):
                load_q(b)
                load_qm(b)
                load_cm(b)
                for p in range(NQ):
                    load_c(b, p)

            # W as [128, 6] columns: a=0,1 -> w_c chunks; 2,3 -> w_q; 4,5 -> w_cq
            w_ps = ps_pt.tile([128, 6], F32, tag="pt")
            nc.tensor.transpose(w_ps[:], wT_raw[:], ident[0:6, 0:6])
            w6 = cpool.tile([128, 6], F32)
            nc.vector.tensor_copy(w6[:], w_ps[:])
            w6r = cpool.tile([128, 6], F32R)
            nc.vector.tensor_copy(w6r[:], w_ps[:])

            # ---------- per-batch derived state ----------
            def setup(b):
                # qT via 4 PE transposes (f32r): cols [dc*256, dc*256+256) hold
                # q rows (j) for d-chunk dc
                qT = bpool.tile([128, 2 * Q_L], F32R, tag="qT")
                for dc in range(2):
                    qt_ps = ps_pt.tile([128, Q_L], F32, tag="pt")
                    for jc in range(2):
                        nc.tensor.transpose(
                            qt_ps[:, jc * 128 : (jc + 1) * 128],
                            q_raws[b][
                                :, jc * D2 + dc * 128 : jc * D2 + (dc + 1) * 128
                            ],
                            ident[:],
                        )
                    nc.scalar.copy(qT[:, dc * Q_L : (dc + 1) * Q_L], qt_ps[:])

                # q chunks (f32r) with a ones column appended (denominator)
                q_ext = []
                for jc in range(2):
                    qe = bpool.tile([128, D2 + 4], F32R, tag=f"q_ext{jc}")
                    nc.vector.tensor_copy(
                        qe[:, 0:D2], q_raws[b][:, jc * D2 : (jc + 1) * D2]
                    )
                    nc.vector.memset(qe[:, D2 : D2 + 1].bitcast(F32), 1.0)
                    nc.vector.memset(qe[:, D2 + 1 : D2 + 4].bitcast(F32), 0.0)
                    q_ext.append(qe)

                # R'[dc] = qT*w_cq + w_c (f32r) ; sq = w_q^T @ qT
                Rp = []
                sq_ps = ps_pt.tile([1, Q_L], F32, tag="pt")
                for dc in range(2):
                    rp = bpool.tile([128, Q_L], F32R, tag=f"Rp{dc}")
                    nc.vector.tensor_scalar(
                        out=rp[:],
                        in0=qT[:, dc * Q_L : (dc + 1) * Q_L],
                        scalar1=w6[:, 4 + dc : 5 + dc],
                        scalar2=w6[:, 0 + dc : 1 + dc],
                        op0=OP.mult,
                        op1=OP.add,
                    )
                    Rp.append(rp)
                    nc.tensor.matmul(
                        sq_ps[:],
                        w6r[:, 2 + dc : 3 + dc],
                        qT[:, dc * Q_L : (dc + 1) * Q_L],
                        start=(dc == 0),
                        stop=(dc == 1),
                    )

                # lng[j] = sq[j] + ln(qm[j]) (ln(0+1e-38) = -87.5 kills masked
                # columns inside the exp); folded into the S0 psum accumulation
                # as a K=1 f32r matmul, so exp(S0+lng) IS masked en
                lnqm = bpool.tile([1, Q_L], F32, tag="lnqm")
                nc.scalar.activation(lnqm[:], qm_rows[b][:], AF.Ln, bias=eps1[:])
                lng = bpool.tile([1, Q_L], F32R, tag="lng")
                nc.vector.tensor_add(lng[:], sq_ps[:], lnqm[:])

                # context mask as [128, 16] (col i = 128-row block i) via PE
                # transpose of the contiguously-loaded [16, 128] layout
                CMps = ps_pt.tile([128, 2 * NP], F32, tag="pt")
                nc.tensor.transpose(CMps[:], cmT_raws[b][:], ident[0:16, 0:16])
                CM = bpool.tile([128, 2 * NP], F32, tag="CM")
                nc.vector.tensor_copy(CM[:], CMps[:])
                E2 = bpool.tile([128, 2 * NP], F32R, tag="E2")

                u2c_ps = ps_u.tile([1, D2], F32, tag="u2c")
                return dict(qT=qT, q_ext=q_ext, Rp=Rp, lng=lng, CM=CM, E2=E2,
                            u2c=u2c_ps)

            # ---------- main loop body: one quad (4 x 128 context rows) -----
            def quad(b, p, st):
                R0 = p * 512
                c_quad = c_quads[b][p]
                en = wpool.tile([128, 1024], F32R, tag="en")
                c2q_quad = wpool.tile([128, 1024], F32, tag="c2q")
                # f32r copy of c for the u2c matmul (walrus requires f32r
                # matmul inputs to be produced rounded); halves on ACT + Pool
                c_r = wpool.tile([128, 1024], F32R, tag="c_r")
                nc.scalar.copy(c_r[:, 0:512], c_quad[:, 0:512])
                nc.gpsimd.tensor_copy(c_r[:, 512:1024], c_quad[:, 512:1024])

                for h in range(2):  # half = pair of context tiles
                    H0 = h * 512
                    # cT: 4 PE transposes -> one psum bank -> sbuf
                    pt_c = ps_pt.tile([128, 512], F32, tag="pt")
                    for o in range(0, 512, 128):
                        nc.tensor.transpose(
                            pt_c[:, o : o + 128],
                            c_quad[:, H0 + o : H0 + o + 128],
                            ident[:],
                        )
                    cT = wpool.tile([128, 512], F32R, tag="cT")
                    if h == 0:
                        nc.vector.tensor_copy(cT[:], pt_c[:])
                    else:
                        nc.scalar.copy(cT[:], pt_c[:])

                    # S0 for both tiles of the pair into one psum bank
                    s0_ps = ps_mm.tile([128, 512], F32, tag="mm")
                    for t in range(2):
                        for dc in range(2):
                            nc.tensor.matmul(
                                s0_ps[:, t * 256 : (t + 1) * 256],
                                cT[:, t * 256 + dc * 128 : t * 256 + (dc + 1) * 128],
                                st["Rp"][dc][:],
                                start=(dc == 0),
                                stop=False,
                            )
                        nc.tensor.matmul(
                            s0_ps[:, t * 256 : (t + 1) * 256],
                            ones_row_r[:],
                            st["lng"][:],
                            start=False,
                            stop=True,
                        )
                    nc.scalar.activation(en[:, H0 : H0 + 512], s0_ps[:], AF.Exp)

                mx = wpool.tile([128, 4], F32, tag="mx", bufs=4)
                for h in range(2):
                    H0 = h * 512
                    # mx = per-tile max over j (en already masked via lng)
                    nc.vector.tensor_reduce(
                        out=mx[:, 2 * h : 2 * h + 2],
                        in_=en[:, H0 : H0 + 512]
                        .bitcast(F32)
                        .rearrange("p (t j) -> p t j", t=2),
                        axis=AX.X,
                        op=OP.max,
                    )
                    nc.vector.tensor_mul(
                        st["E2"][:, 4 * p + 2 * h : 4 * p + 2 * h + 2],
                        mx[:, 2 * h : 2 * h + 2],
                        st["CM"][:, 4 * p + 2 * h : 4 * p + 2 * h + 2],
                    )
                    # enT: 4 PE transposes -> one psum bank -> sbuf (ACT+DVE)
                    pt_e = ps_pt.tile([128, 512], F32, tag="pt")
                    for o in range(0, 512, 128):
                        nc.tensor.transpose(
                            pt_e[:, o : o + 128].bitcast(F32R),
                            en[:, H0 + o : H0 + o + 128],
                            ident_r[:],
                        )
                    enT = wpool.tile([128, 512], F32R, tag="enT")
                    nc.scalar.copy(enT[:, 0:256], pt_e[:, 0:256])
                    nc.vector.tensor_copy(enT[:, 256:512], pt_e[:, 256:512])

                    # c2q per tile (+ denominator in last column); alternate
                    # the normalize-copy between ACT and DVE
                    for t in range(2):
                        c2q_ps = ps_mm.tile([128, D2 + 4], F32, tag="mm")
                        for jc in range(2):
                            nc.tensor.matmul(
                                c2q_ps[:],
                                enT[:, t * 256 + jc * 128 : t * 256 + (jc + 1) * 128],
                                st["q_ext"][jc][:],
                                start=(jc == 0),
                                stop=(jc == 1),
                            )
                        rcp = wpool.tile([128, 1], F32, tag="rcp", bufs=8)
                        nc.vector.reciprocal(rcp[:], c2q_ps[:, D2 : D2 + 1])
                        dst = c2q_quad[:, H0 + t * 256 : H0 + (t + 1) * 256]
                        if t == 0:
                            nc.scalar.activation(
                                dst, c2q_ps[:, 0:D2], AF.Identity, scale=rcp[:]
                            )
                        else:
                            nc.vector.tensor_scalar_mul(dst, c2q_ps[:, 0:D2], rcp[:])

                    # u2c accumulation (q2c numerator), f32r
                    for t in range(2):
                        tt = 2 * h + t
                        nc.tensor.matmul(
                            st["u2c"][:],
                            st["E2"][:, 4 * p + tt : 4 * p + tt + 1],
                            c_r[:, tt * 256 : (tt + 1) * 256],
                            start=(p == 0 and tt == 0),
                            stop=(p == NQ - 1 and tt == 3),
                        )

                # cc2q on gpsimd; stream out G columns 0..768 as quad DMAs
                cc2q = wpool.tile([128, 1024], F32, tag="cc2q")
                for hh in range(2):
                    sl = slice(hh * 512, (hh + 1) * 512)
                    nc.gpsimd.tensor_mul(cc2q[:, sl], c_quad[:, sl], c2q_quad[:, sl])

                for col0, srct in ((0, c_quad), (D2, c2q_quad), (2 * D2, cc2q)):
                    nc.sync.dma_start(
                        out=g_h[b, R0 : R0 + 512, col0 : col0 + D2].rearrange(
                            "(t p) d -> p t d", p=128
                        ),
                        in_=srct[:].rearrange("p (t d) -> p t d", t=4),
                    )

            # ---------- q2c + phase 2 ----------
            def phase2(b, st):
                z2 = bpool.tile([128, 1], F32, tag="z2")
                nc.vector.reduce_sum(z2[:], st["E2"][:].bitcast(F32), axis=AX.X)
                z2_ps = ps_pt.tile([1, 1], F32, tag="pt")
                nc.tensor.matmul(z2_ps[:], z2[:], ones_col[:], start=True, stop=True)
                rz = bpool.tile([1, 1], F32, tag="rz")
                nc.vector.reciprocal(rz[:], z2_ps[:])
                q2c_row = bpool.tile([1, D2], F32R, tag="q2c_row")
                nc.vector.tensor_scalar_mul(q2c_row[:], st["u2c"][:], rz[:])
                q2c_ps = ps_pt.tile([128, D2], F32, tag="pt")
                nc.tensor.matmul(
                    q2c_ps[:], ones_row_r[:], q2c_row[:], start=True, stop=True
                )
                Q2C = bpool.tile([128, D2], F32, tag="Q2C")
                nc.scalar.copy(Q2C[:], q2c_ps[:])

                for p in range(NQ):
                    R0 = p * 512
                    cq2c = wpool.tile([128, 1024], F32, tag="cq2c")
                    for hh in range(2):  # halves run on DVE and Pool in parallel
                        eng = nc.vector if hh == 0 else nc.gpsimd
                        eng.tensor_mul(
                            cq2c[:, hh * 512 : (hh + 1) * 512].rearrange(
                                "p (t d) -> p t d", t=2
                            ),
                            c_quads[b][p][:, hh * 512 : (hh + 1) * 512].rearrange(
                                "p (t d) -> p t d", t=2
                            ),
                            Q2C[:]
                            .rearrange("p (o d) -> p o d", o=1)
                            .broadcast_to([128, 2, D2]),
                        )
                    nc.sync.dma_start(
                        out=g_h[b, R0 : R0 + 512, 3 * D2 : 4 * D2].rearrange(
                            "(t p) d -> p t d", p=128
                        ),
                        in_=cq2c[:].rearrange("p (t d) -> p t d", t=4),
                    )

            # ---------- drive: next batch's setup is emitted BEFORE this
            # batch's phase 2 so engines always have independent queued work
            # across the z2 -> Q2C dependency chain ----------
            st = setup(0)
            for b in range(BPC):
                cur = st
                for p in range(NQ):
                    quad(b, p, cur)
                if b + 1 < BPC:
                    st = setup(b + 1)
                phase2(b, cur)

    _spill_excess_waits(nc)
    return nc


_NC_CACHE = None


def _get_nc():
    global _NC_CACHE
    if _NC_CACHE is None:
        _NC_CACHE = build_bass()
    return _NC_CACHE


def kernel(**inputs) -> np.ndarray:
    ctx = np.ascontiguousarray(np.asarray(inputs["context"], dtype=np.float32))
    cm = np.ascontiguousarray(np.asarray(inputs["context_mask"], dtype=np.float32))
    q = np.ascontiguousarray(np.asarray(inputs["query"], dtype=np.float32))
    qm = np.ascontiguousarray(np.asarray(inputs["query_mask"], dtype=np.float32))
    w = np.ascontiguousarray(np.asarray(inputs["W"], dtype=np.float32))

    in_maps = []
    for core in range(N_CORES):
        lo, hi = core * BPC, (core + 1) * BPC
        in_maps.append(
            {
                "context": ctx[lo:hi],
                "context_mask": cm[lo:hi],
                "query": q[lo:hi],
                "query_mask": qm[lo:hi],
                "W": w,
            }
        )

    nc = _get_nc()
    res = run_bass_kernel_spmd(nc, in_maps, list(range(N_CORES)))
    return np.concatenate([res.results[i]["G"] for i in range(N_CORES)], axis=0)
